# revision 2
# baseline (speedup 1.0000x reference)
"""Causal self-attention (B=4, S=2048, D=1024, H=16) on 8 TRN2 NeuronCores, v2.

Sharding: core c -> batch b = c//2, head-group g = c%2 (8 heads each).

v2 design (vs baseline): all matmul operands bf16; attention-value product in
natural [q, f] layout (65 -> 64+1 rows per block via separate 1-row denominator
matmuls, halving AV tensor time); exp over 2-chunk score pairs with shifted
packing on diagonal pairs; causal triangle masked by a Pool tensor_tensor over
both triangle blocks of a pair; y transposed back to feature-major via PE
transposes; out-proj PSUM->SBUF moves on the Activation engine; stage1(j+1) and
outproj(j-1) matmuls woven between score/AV work to keep PE busy.

Layouts (per core):
  xT    [128, 8, 2048] bf16   xT[p,t,s] = x[b][s, t*128+p]
  wqk   [128, 8, 8, 128] bf16 q/k projection, f-tile-major
  wvd   [128, 8, 512] bf16    v projection
  wod   [128, 8, 4, 128] bf16 out projection (f-major partitions)
  qt/kt [128, 4, 512] bf16    feature-major q/k; partitions = 2 heads x 64
  vp    [128, 4, 512] bf16    natural v [s, f] per 128-seq chunk
  e     [128, 1024] bf16      exp(scores^T) pair tiles [k=128, q up to 2x512]
  psy   [128, 4, 2, 64] f32   natural y accum per m-pair (1 PSUM bank)
  pd    [128, 32] f32         denominators (m, half, subchunk)
  yT    [128, 4, 512] bf16    feature-major y for out-proj
"""

import numpy as np

import concourse.bacc as bacc
import concourse.mybir as mybir
from concourse.tile import TileContext
from concourse.bass_utils import run_bass_kernel_spmd

BF16 = mybir.dt.bfloat16
F32 = mybir.dt.float32
AF = mybir.ActivationFunctionType
OP = mybir.AluOpType

B, S, D = 4, 2048, 1024
H = 16
HD = 64
HL = 8          # heads per core
SB = 512        # sequence block
NJ = S // SB    # 4 s-blocks
DT = D // 128   # 8 contraction tiles

_CACHED_NC = None


def build_nc():
    nc = bacc.Bacc(None, target_bir_lowering=False)

    xT = nc.dram_tensor("xT", [128, DT, S], BF16, kind="ExternalInput")
    wqk = nc.dram_tensor("wqk", [128, 8, DT, 128], BF16, kind="ExternalInput")
    wvd = nc.dram_tensor("wvd", [128, DT, 512], BF16, kind="ExternalInput")
    wod = nc.dram_tensor("wod", [128, 8, 4, 128], BF16, kind="ExternalInput")
    bqkv = nc.dram_tensor("bqkv", [128, 12], F32, kind="ExternalInput")
    bv = nc.dram_tensor("bv", [1, 512], BF16, kind="ExternalInput")
    tri = nc.dram_tensor("tri", [128, 128], BF16, kind="ExternalInput")
    ident = nc.dram_tensor("ident", [128, 128], BF16, kind="ExternalInput")
    out = nc.dram_tensor("out_part", [8, 128, S], BF16, kind="ExternalOutput")

    with TileContext(nc) as tc:
        with (
            tc.tile_pool(name="const", bufs=1) as cpool,
            tc.tile_pool(name="big", bufs=1) as bpool,
            tc.tile_pool(name="qq", bufs=2) as qpool,
            tc.tile_pool(name="kk", bufs=4) as kpool,
            tc.tile_pool(name="vv", bufs=4) as vpool,
            tc.tile_pool(name="xb", bufs=2) as xpool,
            tc.tile_pool(name="ee", bufs=36) as epool,
            tc.tile_pool(name="yn", bufs=3) as ynpool,
            tc.tile_pool(name="rd", bufs=2) as rdpool,
            tc.tile_pool(name="yt", bufs=3) as ypool,
            tc.tile_pool(name="ob", bufs=3) as opool,
            tc.tile_pool(name="p1", bufs=2, space="PSUM") as p1,
            tc.tile_pool(name="pscore", bufs=2, space="PSUM") as pscore,
            tc.tile_pool(name="py", bufs=1, space="PSUM") as py,
            tc.tile_pool(name="paux", bufs=1, space="PSUM") as pauxpool,
        ):
            # ---- constants (tiles now; DMAs emitted after the first x/w
            # loads so the hot path owns the DMA queues at startup) ----
            tri_t = cpool.tile([128, 128], BF16)
            ident_t = cpool.tile([128, 128], BF16)
            bqkv_t = cpool.tile([128, 12], F32)
            bv_t = cpool.tile([1, 512], BF16)
            bvb = cpool.tile([128, 512], F32)
            ones = cpool.tile([128, 1], BF16)
            nc.vector.memset(ones[:], 1.0)
            ones_row = cpool.tile([1, 128], BF16)
            nc.vector.memset(ones_row[:], 1.0)

            wv = bpool.tile([128, DT, 512], BF16)
            wod_t = bpool.tile([128, 8, 4, 128], BF16)
            wqk_t = bpool.tile([128, 8, DT, 128], BF16)

            xblk = {}
            qk_blk = {}
            v_blk = {}
            yts = {}

            # one PSUM bank shared by denominators (cols 0:32 f32) and
            # rotating transpose-output regions (3 x [128,128] bf16)
            aux = pauxpool.tile([128, 512], F32)
            pt_ctr = [0]

            def pt_region():
                r = pt_ctr[0] % 3
                pt_ctr[0] += 1
                lo = 32 + r * 64
                return aux[:, lo:lo + 64].bitcast(BF16)

            # ---- stage 1 pieces (emitted interleaved with attention) ----
            def s1_load(j, split=False):
                xb = xpool.tile([128, DT, SB], BF16)
                for d in range(DT):  # split so first matmul starts early
                    eng = nc.scalar if (split and d % 2) else nc.sync
                    eng.dma_start(xb[:, d, :], xT[:, d, j * SB:(j + 1) * SB])
                qt = qpool.tile([128, 4, SB], BF16)
                kt = kpool.tile([128, 4, SB], BF16)
                qk_blk[j] = kt
                xblk[j] = (xb, qt, kt)

            def s1_f(j, f):
                xb, qt, kt = xblk[j]
                ps = p1.tile([128, SB], F32, tag="ps")
                for d in range(DT):
                    nc.tensor.matmul(
                        ps[:], wqk_t[:, f, d, :], xb[:, d, :],
                        start=(d == 0), stop=(d == DT - 1),
                    )
                dst = qt[:, f, :] if f < 4 else kt[:, f - 4, :]
                nc.vector.tensor_scalar(
                    dst, ps[:], bqkv_t[:, f:f + 1], None, OP.add
                )
                emitted_f.add((j, f))

            def s1_v(j, s4):
                xb = xblk[j][0]
                if s4 == 0:
                    vp = vpool.tile([128, 4, 512], BF16)
                    v_blk[j] = vp
                vp = v_blk[j]
                ps = p1.tile([128, 512], F32, tag="ps")
                for d in range(DT):
                    nc.tensor.matmul(
                        ps[:], xb[:, d, s4 * 128:(s4 + 1) * 128], wv[:, d, :],
                        start=(d == 0), stop=(d == DT - 1),
                    )
                nc.vector.tensor_tensor(vp[:, s4, :], ps[:], bvb[:], OP.add)

            def outproj(j, t, tail=False):
                yT = yts[j]
                ps = p1.tile([128, SB], F32, tag="ps")
                for ff in range(4):
                    nc.tensor.matmul(
                        ps[:], wod_t[:, t, ff, :], yT[:, ff, :],
                        start=(ff == 0), stop=(ff == 3),
                    )
                ob = opool.tile([128, SB], BF16)
                nc.vector.tensor_copy(ob[:], ps[:])
                eng = nc.scalar if (j == NJ - 1 and t % 2) else nc.sync
                eng.dma_start(out[t, :, j * SB:(j + 1) * SB], ob[:])

            # ---- filler management ----
            fillers = []
            pending_t = []  # deferred transposes (keep newest 1 in flight)
            epair_map = {}
            emitted_f = set()
            tail_mode = [False]

            def pop_filler(k=1, no_pending=False):
                for _ in range(k):
                    if len(pending_t) > 1 and not no_pending:
                        pending_t.pop(0)()
                    elif fillers:
                        fillers.pop(0)()

            def flush_t():
                while pending_t:
                    pending_t.pop(0)()

            # ---- attention per (j, m): score units + A-phase ----
            def s_unit(j, m, p, e_pairs):
                """Scores + exp for one k-chunk pair of head-pair m, block j.

                e_pairs[(half, p)] = (e_tile, [col offset of chunk 2p, 2p+1]).
                Diagonal chunks (global chunk index i >= 4j) are written
                column-shifted so each pair's valid region is contiguous
                from its slot start; exp then covers exactly the valid span
                (plus any gap between the two chunks' slots, never read).
                """
                qk = xblk[j][1]
                diag = 2 * p >= 4 * j
                if True:
                    for half in range(2):
                        b0 = half * 64
                        ps = pscore.tile([128, 1024], F32)
                        e = epool.tile([128, 1024], BF16)
                        offs = []
                        base = 0
                        tri_offs = []
                        for q in range(2):
                            i = 2 * p + q
                            ib, il = i // 4, i % 4
                            c0 = max(0, i * 128 - j * SB)
                            n = SB - c0
                            # shifted packing: valid q-range [c0, 512) lands
                            # at cols [base, base + n); slot must not cross a
                            # 512-col PSUM bank boundary
                            if q == 1 and base + n > 512:
                                base = 512
                            ks = qk_blk[ib][b0:b0 + 64, m,
                                            il * 128:il * 128 + 128]
                            qs = qk[b0:b0 + 64, m, c0:SB]
                            nc.tensor.matmul(
                                ps[:, base:base + n], ks, qs,
                                start=True, stop=True,
                            )
                            offs.append(base)
                            if diag:
                                tri_offs.append(base)
                            base += n
                        # one exp over the union [0, end of odd slot)
                        nc.scalar.activation(
                            e[:, :base], ps[:, :base], AF.Exp, scale=0.125
                        )
                        for to in tri_offs:  # mask causal triangles
                            nc.gpsimd.tensor_tensor(
                                e[:, to:to + 128], e[:, to:to + 128],
                                tri_t[:], OP.mult,
                            )
                        e_pairs[(half, p)] = (e, offs)
                    if p % 2 == 0 or j < 2:
                        pop_filler(1)

            def s_units(j, m):
                """Per-pair emission thunks, diagonal pairs first: the
                A-phase consumes every pair in its very first s-chain, so the
                last-produced pair must not be the one with the longest
                exp+mask latency."""
                e_pairs = {}
                epair_map[(j, m)] = e_pairs
                n_sk = 4 * (j + 1)
                order = list(range(2 * j, n_sk // 2)) + list(range(2 * j))
                return [
                    (lambda pp=p: s_unit(j, m, pp, e_pairs)) for p in order
                ]

            def a_phase(j, m, e_pairs, psy, yn, feed=()):
                """AV in natural [q, f] layout + denominators + normalize +
                transpose into yT[j][:, m, :]."""
                yT = yts[j]
                for s in range(4):
                    n_i = 4 * j + s + 1
                    for half in range(2):
                        h = 2 * m + half
                        for i in range(n_i):
                            ib, il = i // 4, i % 4
                            c0 = max(0, i * 128 - j * SB)
                            e, offs = e_pairs[(half, i // 2)]
                            col = offs[i % 2] + s * 128 - c0
                            el = e[:, col:col + 128]
                            vs = v_blk[ib][:, il, h * 64:h * 64 + 64]
                            nc.tensor.matmul(
                                psy[:, s, half, :], el, vs,
                                start=(i == 0), stop=(i == n_i - 1),
                            )
                            dcol = (m * 2 + half) * 4 + s
                            nc.tensor.matmul(
                                aux[:, dcol:dcol + 1], el, ones[:],
                                start=(i == 0), stop=(i == n_i - 1),
                            )
                    # feed the next (j, m)'s score pairs between AV chains
                    # so the Activation engine never drains at m boundaries
                    for u in feed[2 * s:2 * s + 2]:
                        u()
                    pop_filler(1)
                # normalize after all AV chains: hazards are tracked per
                # tile, so touching psy/aux between chains would stall the
                # next chain's accumulation. DVE has no divide: recip + mult.
                rd = rdpool.tile([128, 8], F32)
                nc.vector.reciprocal(rd[:], aux[:, m * 8:m * 8 + 8])
                for s in range(4):
                    for half in range(2):
                        dcol = half * 4 + s
                        nc.vector.tensor_scalar(
                            yn[:, s, half * 64:half * 64 + 64],
                            psy[:, s, half, :],
                            rd[:, dcol:dcol + 1], None, OP.mult,
                        )

                    def transp(mm=m, ss=s, yy=yn):
                        ptb = pt_region()
                        nc.tensor.transpose(ptb, yy[:, ss, :], ident_t[:])
                        nc.vector.tensor_copy(
                            yT[:, mm, ss * 128:ss * 128 + 128], ptb)

                    # defer: this s-group's divide is still in flight on DVE;
                    # running the transpose now would stall PE
                    pending_t.append(transp)

            # ---- main pipeline ----
            nc.scalar.dma_start(wqk_t[:, 0], wqk[:, 0])
            s1_load(0, split=True)
            nc.sync.dma_start(bqkv_t[:], bqkv[:])
            nc.sync.dma_start(bv_t[:], bv[:])
            nc.sync.dma_start(wqk_t[:, 4], wqk[:, 4])
            s1_f(0, 0)
            nc.sync.dma_start(wv[:], wvd[:])
            nc.sync.dma_start(tri_t[:], tri[:])
            nc.sync.dma_start(ident_t[:], ident[:])
            for f in (1, 2, 3, 5, 6, 7):
                nc.sync.dma_start(wqk_t[:, f], wqk[:, f])
            s1_f(0, 4)
            # broadcast bv to all partitions via a rank-1 matmul (cheaper
            # than a gpsimd partition_broadcast; placed here so the wait on
            # the bv DMA is hidden behind the first two qk chains)
            ps_b = p1.tile([128, 512], F32, tag="ps")
            nc.tensor.matmul(ps_b[:], ones_row[:], bv_t[:], start=True,
                             stop=True)
            nc.vector.tensor_copy(bvb[:], ps_b[:])
            s1_f(0, 1)
            s1_f(0, 5)
            for s4 in range(4):
                s1_v(0, s4)
            s1_load(1)
            nc.sync.dma_start(wod_t[:], wod[:])

            def ensure_f(jj, mm):
                """Pop fillers until q/k f-tiles (mm, mm+4) of block jj have
                been emitted (score units for (jj, mm) read them)."""
                guard = len(fillers) + 4
                while not ({(jj, mm), (jj, mm + 4)} <= emitted_f):
                    guard -= 1
                    assert guard > 0, f"f-tiles ({jj},{mm}) never queued"
                    pop_filler(1, no_pending=True)

            # seed the stream with (0, 0) scores
            for u in s_units(0, 0):
                u()

            for j in range(NJ):
                if j + 1 < NJ:
                    # q/k f-tiles for m=0 plus v(j+1) must land during j; the
                    # later m f-tiles are deferred into j+1 itself so its
                    # attention phases (Act-heavy) keep PE fed
                    fillers.extend(
                        [lambda jj=j + 1, ff=f: s1_f(jj, ff) for f in (0, 4)]
                    )
                    fillers.extend(
                        [lambda jj=j + 1, ss=s4: s1_v(jj, ss)
                         for s4 in range(4)]
                    )
                    if j + 2 < NJ:
                        fillers.append(lambda jj=j + 2: s1_load(jj))
                if j == 2:  # outproj(0) deferred to here
                    fillers.extend(
                        [lambda tt=t: outproj(0, tt) for t in range(8)]
                    )
                elif j == 3:  # outproj(1) and (2) as j=3 fillers
                    fillers.extend(
                        [lambda tt=t: outproj(1, tt) for t in range(8)]
                    )
                    fillers.extend(
                        [lambda tt=t: outproj(2, tt) for t in range(8)]
                    )
                yT = ypool.tile([128, 4, SB], BF16)
                yts[j] = yT
                for m in range(4):
                    # this j's own later q/k f-tiles, just in time
                    # (j=0 built f-tiles 1,5 upfront already)
                    mo = m + 2 if j == 0 else m + 1
                    if mo < 4:
                        fillers.insert(0, lambda jj=j, ff=mo + 4: s1_f(jj, ff))
                        fillers.insert(0, lambda jj=j, ff=mo: s1_f(jj, ff))
                    psy = py.tile([128, 4, 2, 64], F32)
                    yn = ynpool.tile([128, 4, 128], BF16)
                    # next (j, m)'s scores stream inside this A-phase; hold
                    # back the last 2 pairs so the next A-phase's first AV
                    # does not collide with this m's psy normalizes
                    if (j, m) == (NJ - 1, 3):
                        feed, post = [], []
                        tail_mode[0] = True
                    else:
                        jn, mn = (j, m + 1) if m < 3 else (j + 1, 0)
                        ensure_f(jn, mn)
                        un = s_units(jn, mn)
                        feed, post = un[:max(0, len(un) - 2)], un[-2:]
                    a_phase(j, m, epair_map[(j, m)], psy, yn, feed)
                    for u in post:
                        u()
                # drain this j's fillers before moving on
                flush_t()
                pop_filler(len(fillers))
            for t in range(8):
                outproj(NJ - 1, t, tail=True)

    nc.finalize()
    return nc


def _to_bf16(a):
    import ml_dtypes
    return np.ascontiguousarray(a).astype(ml_dtypes.bfloat16)


def _prep_core_inputs(x, w_qkv, b_qkv, w_out, core):
    b = core // 2
    g = core % 2
    rows = np.concatenate([
        w_qkv[512 * g:512 * g + 512],
        w_qkv[1024 + 512 * g:1024 + 512 * g + 512],
        w_qkv[2048 + 512 * g:2048 + 512 * g + 512],
    ], axis=0)  # [1536, 1024]
    brows = np.concatenate([
        b_qkv[512 * g:512 * g + 512],
        b_qkv[1024 + 512 * g:1024 + 512 * g + 512],
        b_qkv[2048 + 512 * g:2048 + 512 * g + 512],
    ])  # [1536]

    xT = np.ascontiguousarray(
        x[b].T.reshape(DT, 128, S).transpose(1, 0, 2)
    )
    rT = rows.T.reshape(DT, 128, 12, 128)          # [d, p, ftile, fc]
    wqk = np.ascontiguousarray(rT[:, :, :8].transpose(1, 2, 0, 3))
    wvd = np.ascontiguousarray(
        rows.T.reshape(DT, 128, 1536)[:, :, 1024:].transpose(1, 0, 2)
    )
    ws = w_out[:, 512 * g:512 * g + 512]           # [do(1024), f(512)]
    wod = np.ascontiguousarray(
        ws.reshape(8, 128, 4, 128).transpose(3, 0, 2, 1))
    bqkv = np.ascontiguousarray(
        brows[:1536].reshape(12, 128).T).astype(np.float32)
    bvb = _to_bf16(brows[1024:1536].reshape(1, 512))
    tri = (np.arange(128)[:, None] <= np.arange(128)[None, :]).astype(
        np.float32)
    ident = np.eye(128, dtype=np.float32)

    return {
        "xT": _to_bf16(xT), "wqk": _to_bf16(wqk), "wvd": _to_bf16(wvd),
        "wod": _to_bf16(wod), "bqkv": bqkv, "bv": bvb,
        "tri": _to_bf16(tri), "ident": _to_bf16(ident),
    }


def kernel(x, w_qkv, b_qkv, w_out, b_out):
    global _CACHED_NC
    x = np.asarray(x, dtype=np.float32)
    w_qkv = np.asarray(w_qkv, dtype=np.float32)
    b_qkv = np.asarray(b_qkv, dtype=np.float32)
    w_out = np.asarray(w_out, dtype=np.float32)
    b_out = np.asarray(b_out, dtype=np.float32)

    if _CACHED_NC is None:
        _CACHED_NC = build_nc()
    nc = _CACHED_NC

    in_maps = [
        _prep_core_inputs(x, w_qkv, b_qkv, w_out, c) for c in range(8)
    ]
    last_err = None
    for attempt in range(5):
        try:
            res = run_bass_kernel_spmd(nc, in_maps, core_ids=list(range(8)))
            break
        except Exception as e:  # transient NRT/axon wedge: retry
            last_err = e
            import time
            time.sleep(20)
    else:
        raise last_err

    out = np.empty((B, S, D), dtype=np.float32)
    for b in range(B):
        p0 = np.asarray(res.results[2 * b]["out_part"]).astype(np.float32)
        p1_ = np.asarray(res.results[2 * b + 1]["out_part"]).astype(np.float32)
        tot = (p0 + p1_).reshape(D, S)  # [do, s]
        out[b] = tot.T + b_out[None, :]
    return out


# revision 3
# speedup vs baseline: 1.0091x; 1.0091x over previous
"""Causal self-attention (B=4, S=2048, D=1024, H=16) on 8 TRN2 NeuronCores, v2.

Sharding: core c -> batch b = c//2, head-group g = c%2 (8 heads each).

v2 design (vs baseline): all matmul operands bf16; attention-value product in
natural [q, f] layout (65 -> 64+1 rows per block via separate 1-row denominator
matmuls, halving AV tensor time); exp over 2-chunk score pairs with shifted
packing on diagonal pairs; causal triangle masked by a Pool tensor_tensor over
both triangle blocks of a pair; y transposed back to feature-major via PE
transposes; out-proj PSUM->SBUF moves on the Activation engine; stage1(j+1) and
outproj(j-1) matmuls woven between score/AV work to keep PE busy.

Layouts (per core):
  xT    [128, 8, 2048] bf16   xT[p,t,s] = x[b][s, t*128+p]
  wqk   [128, 8, 8, 128] bf16 q/k projection, f-tile-major
  wvd   [128, 8, 512] bf16    v projection
  wod   [128, 8, 4, 128] bf16 out projection (f-major partitions)
  qt/kt [128, 4, 512] bf16    feature-major q/k; partitions = 2 heads x 64
  vp    [128, 4, 512] bf16    natural v [s, f] per 128-seq chunk
  e     [128, 1024] bf16      exp(scores^T) pair tiles [k=128, q up to 2x512]
  psy   [128, 4, 2, 64] f32   natural y accum per m-pair (1 PSUM bank)
  pd    [128, 32] f32         denominators (m, half, subchunk)
  yT    [128, 4, 512] bf16    feature-major y for out-proj
"""

import numpy as np

import concourse.bacc as bacc
import concourse.mybir as mybir
from concourse.tile import TileContext
from concourse.bass_utils import run_bass_kernel_spmd

BF16 = mybir.dt.bfloat16
F32 = mybir.dt.float32
AF = mybir.ActivationFunctionType
OP = mybir.AluOpType

B, S, D = 4, 2048, 1024
H = 16
HD = 64
HL = 8          # heads per core
SB = 512        # sequence block
NJ = S // SB    # 4 s-blocks
DT = D // 128   # 8 contraction tiles

_CACHED_NC = None


def build_nc():
    nc = bacc.Bacc(None, target_bir_lowering=False)

    xT = nc.dram_tensor("xT", [128, DT, S], BF16, kind="ExternalInput")
    wqk = nc.dram_tensor("wqk", [128, 8, DT, 128], BF16, kind="ExternalInput")
    wvd = nc.dram_tensor("wvd", [128, DT, 512], BF16, kind="ExternalInput")
    wod = nc.dram_tensor("wod", [128, 8, 4, 128], BF16, kind="ExternalInput")
    bqkv = nc.dram_tensor("bqkv", [128, 12], F32, kind="ExternalInput")
    bv = nc.dram_tensor("bv", [1, 512], BF16, kind="ExternalInput")
    tri = nc.dram_tensor("tri", [128, 128], BF16, kind="ExternalInput")
    ident = nc.dram_tensor("ident", [128, 128], BF16, kind="ExternalInput")
    out = nc.dram_tensor("out_part", [8, 128, S], BF16, kind="ExternalOutput")

    with TileContext(nc) as tc:
        with (
            tc.tile_pool(name="const", bufs=1) as cpool,
            tc.tile_pool(name="big", bufs=1) as bpool,
            tc.tile_pool(name="qq", bufs=2) as qpool,
            tc.tile_pool(name="kk", bufs=4) as kpool,
            tc.tile_pool(name="vv", bufs=4) as vpool,
            tc.tile_pool(name="xb", bufs=2) as xpool,
            tc.tile_pool(name="ee", bufs=36) as epool,
            tc.tile_pool(name="yn", bufs=3) as ynpool,
            tc.tile_pool(name="rd", bufs=2) as rdpool,
            tc.tile_pool(name="yt", bufs=3) as ypool,
            tc.tile_pool(name="ob", bufs=3) as opool,
            tc.tile_pool(name="p1", bufs=2, space="PSUM") as p1,
            tc.tile_pool(name="pscore", bufs=2, space="PSUM") as pscore,
            tc.tile_pool(name="py", bufs=1, space="PSUM") as py,
            tc.tile_pool(name="paux", bufs=1, space="PSUM") as pauxpool,
        ):
            # ---- constants (tiles now; DMAs emitted after the first x/w
            # loads so the hot path owns the DMA queues at startup) ----
            tri_t = cpool.tile([128, 128], BF16)
            ident_t = cpool.tile([128, 128], BF16)
            bqkv_t = cpool.tile([128, 12], F32)
            bv_t = cpool.tile([1, 512], BF16)
            bvb = cpool.tile([128, 512], F32)
            ones = cpool.tile([128, 1], BF16)
            nc.vector.memset(ones[:], 1.0)
            ones_row = cpool.tile([1, 128], BF16)
            nc.vector.memset(ones_row[:], 1.0)

            wv = bpool.tile([128, DT, 512], BF16)
            wod_t = bpool.tile([128, 8, 4, 128], BF16)
            wqk_t = bpool.tile([128, 8, DT, 128], BF16)

            xblk = {}
            qk_blk = {}
            v_blk = {}
            yts = {}

            # one PSUM bank shared by denominators (cols 0:32 f32) and
            # rotating transpose-output regions (3 x [128,128] bf16)
            aux = pauxpool.tile([128, 512], F32)
            pt_ctr = [0]

            def pt_region():
                r = pt_ctr[0] % 3
                pt_ctr[0] += 1
                lo = 32 + r * 64
                return aux[:, lo:lo + 64].bitcast(BF16)

            # ---- stage 1 pieces (emitted interleaved with attention) ----
            def s1_load(j, split=False):
                xb = xpool.tile([128, DT, SB], BF16)
                for d in range(DT):  # split so first matmul starts early
                    eng = nc.scalar if (split and d % 2) else nc.sync
                    eng.dma_start(xb[:, d, :], xT[:, d, j * SB:(j + 1) * SB])
                qt = qpool.tile([128, 4, SB], BF16)
                kt = kpool.tile([128, 4, SB], BF16)
                qk_blk[j] = kt
                xblk[j] = (xb, qt, kt)

            def s1_f(j, f):
                xb, qt, kt = xblk[j]
                ps = p1.tile([128, SB], F32, tag="ps")
                for d in range(DT):
                    nc.tensor.matmul(
                        ps[:], wqk_t[:, f, d, :], xb[:, d, :],
                        start=(d == 0), stop=(d == DT - 1),
                    )
                dst = qt[:, f, :] if f < 4 else kt[:, f - 4, :]
                nc.vector.tensor_scalar(
                    dst, ps[:], bqkv_t[:, f:f + 1], None, OP.add
                )
                emitted_f.add((j, f))

            def s1_v(j, s4):
                xb = xblk[j][0]
                if s4 == 0:
                    vp = vpool.tile([128, 4, 512], BF16)
                    v_blk[j] = vp
                vp = v_blk[j]
                ps = p1.tile([128, 512], F32, tag="ps")
                for d in range(DT):
                    nc.tensor.matmul(
                        ps[:], xb[:, d, s4 * 128:(s4 + 1) * 128], wv[:, d, :],
                        start=(d == 0), stop=(d == DT - 1),
                    )
                nc.vector.tensor_tensor(vp[:, s4, :], ps[:], bvb[:], OP.add)

            def outproj(j, t, tail=False):
                yT = yts[j]
                ps = p1.tile([128, SB], F32, tag="ps")
                for ff in range(4):
                    nc.tensor.matmul(
                        ps[:], wod_t[:, t, ff, :], yT[:, ff, :],
                        start=(ff == 0), stop=(ff == 3),
                    )
                ob = opool.tile([128, SB], BF16)
                nc.vector.tensor_copy(ob[:], ps[:])
                eng = nc.scalar if (j == NJ - 1 and t % 2) else nc.sync
                eng.dma_start(out[t, :, j * SB:(j + 1) * SB], ob[:])

            # ---- filler management ----
            fillers = []
            pending_t = []  # deferred transposes (keep newest 1 in flight)
            epair_map = {}
            emitted_f = set()
            tail_mode = [False]

            def pop_filler(k=1, no_pending=False):
                for _ in range(k):
                    if len(pending_t) > 1 and not no_pending:
                        pending_t.pop(0)()
                    elif fillers:
                        fillers.pop(0)()

            def flush_t():
                while pending_t:
                    pending_t.pop(0)()

            # ---- attention per (j, m): score units + A-phase ----
            def s_unit(j, m, p, e_pairs):
                """Scores + exp for one k-chunk pair of head-pair m, block j.

                e_pairs[(half, p)] = (e_tile, [col offset of chunk 2p, 2p+1]).
                Diagonal chunks (global chunk index i >= 4j) are written
                column-shifted so each pair's valid region is contiguous
                from its slot start; exp then covers exactly the valid span
                (plus any gap between the two chunks' slots, never read).
                """
                qk = xblk[j][1]
                diag = 2 * p >= 4 * j
                if True:
                    for half in range(2):
                        b0 = half * 64
                        ps = pscore.tile([128, 1024], F32)
                        e = epool.tile([128, 1024], BF16)
                        offs = []
                        base = 0
                        tri_offs = []
                        for q in range(2):
                            i = 2 * p + q
                            ib, il = i // 4, i % 4
                            c0 = max(0, i * 128 - j * SB)
                            n = SB - c0
                            # shifted packing: valid q-range [c0, 512) lands
                            # at cols [base, base + n); slot must not cross a
                            # 512-col PSUM bank boundary
                            if q == 1 and base + n > 512:
                                base = 512
                            ks = qk_blk[ib][b0:b0 + 64, m,
                                            il * 128:il * 128 + 128]
                            qs = qk[b0:b0 + 64, m, c0:SB]
                            nc.tensor.matmul(
                                ps[:, base:base + n], ks, qs,
                                start=True, stop=True,
                            )
                            offs.append(base)
                            if diag:
                                tri_offs.append(base)
                            base += n
                        # one exp over the union [0, end of odd slot)
                        nc.scalar.activation(
                            e[:, :base], ps[:, :base], AF.Exp, scale=0.125
                        )
                        for to in tri_offs:  # mask causal triangles
                            nc.gpsimd.tensor_tensor(
                                e[:, to:to + 128], e[:, to:to + 128],
                                tri_t[:], OP.mult,
                            )
                        e_pairs[(half, p)] = (e, offs)
                    if p % 2 == 0 or j < 2:
                        pop_filler(1)

            def s_units(j, m):
                """Per-pair emission thunks, diagonal pairs first: the
                A-phase consumes every pair in its very first s-chain, so the
                last-produced pair must not be the one with the longest
                exp+mask latency."""
                e_pairs = {}
                epair_map[(j, m)] = e_pairs
                n_sk = 4 * (j + 1)
                order = list(range(2 * j, n_sk // 2)) + list(range(2 * j))
                return [
                    (lambda pp=p: s_unit(j, m, pp, e_pairs)) for p in order
                ]

            def a_phase(j, m, e_pairs, psy, yn, feed=()):
                """AV in natural [q, f] layout + denominators + normalize +
                transpose into yT[j][:, m, :]."""
                yT = yts[j]
                for s in range(4):
                    n_i = 4 * j + s + 1
                    for half in range(2):
                        h = 2 * m + half
                        for i in range(n_i):
                            ib, il = i // 4, i % 4
                            c0 = max(0, i * 128 - j * SB)
                            e, offs = e_pairs[(half, i // 2)]
                            col = offs[i % 2] + s * 128 - c0
                            el = e[:, col:col + 128]
                            vs = v_blk[ib][:, il, h * 64:h * 64 + 64]
                            nc.tensor.matmul(
                                psy[:, s, half, :], el, vs,
                                start=(i == 0), stop=(i == n_i - 1),
                            )
                            dcol = (m * 2 + half) * 4 + s
                            nc.tensor.matmul(
                                aux[:, dcol:dcol + 1], el, ones[:],
                                start=(i == 0), stop=(i == n_i - 1),
                            )
                    # feed the next (j, m)'s score pairs between AV chains
                    # so the Activation engine never drains at m boundaries
                    for u in feed[2 * s:2 * s + 2]:
                        u()
                    pop_filler(1)
                # normalize after all AV chains: hazards are tracked per
                # tile, so touching psy/aux between chains would stall the
                # next chain's accumulation. DVE has no divide: recip + mult.
                rd = rdpool.tile([128, 8], F32)
                nc.vector.reciprocal(rd[:], aux[:, m * 8:m * 8 + 8])
                for s in range(4):
                    for half in range(2):
                        dcol = half * 4 + s
                        nc.vector.tensor_scalar(
                            yn[:, s, half * 64:half * 64 + 64],
                            psy[:, s, half, :],
                            rd[:, dcol:dcol + 1], None, OP.mult,
                        )

                    def transp(mm=m, ss=s, yy=yn):
                        ptb = pt_region()
                        nc.tensor.transpose(ptb, yy[:, ss, :], ident_t[:])
                        nc.vector.tensor_copy(
                            yT[:, mm, ss * 128:ss * 128 + 128], ptb)

                    # defer: this s-group's divide is still in flight on DVE;
                    # running the transpose now would stall PE
                    pending_t.append(transp)

            # ---- main pipeline ----
            nc.scalar.dma_start(wqk_t[:, 0], wqk[:, 0])
            s1_load(0, split=True)
            nc.gpsimd.dma_start(bqkv_t[:], bqkv[:])
            nc.gpsimd.dma_start(bv_t[:], bv[:])
            nc.sync.dma_start(wqk_t[:, 4], wqk[:, 4])
            s1_f(0, 0)
            nc.sync.dma_start(wv[:], wvd[:])
            nc.gpsimd.dma_start(tri_t[:], tri[:])
            nc.gpsimd.dma_start(ident_t[:], ident[:])
            for f in (1, 2, 3, 5, 6, 7):
                nc.sync.dma_start(wqk_t[:, f], wqk[:, f])
            s1_f(0, 4)
            # broadcast bv to all partitions via a rank-1 matmul (cheaper
            # than a gpsimd partition_broadcast; placed here so the wait on
            # the bv DMA is hidden behind the first two qk chains)
            ps_b = p1.tile([128, 512], F32, tag="ps")
            nc.tensor.matmul(ps_b[:], ones_row[:], bv_t[:], start=True,
                             stop=True)
            nc.vector.tensor_copy(bvb[:], ps_b[:])
            s1_f(0, 1)
            s1_f(0, 5)
            for s4 in range(4):
                s1_v(0, s4)
            s1_load(1)
            nc.sync.dma_start(wod_t[:], wod[:])

            def ensure_f(jj, mm):
                """Pop fillers until q/k f-tiles (mm, mm+4) of block jj have
                been emitted (score units for (jj, mm) read them)."""
                guard = len(fillers) + 4
                while not ({(jj, mm), (jj, mm + 4)} <= emitted_f):
                    guard -= 1
                    assert guard > 0, f"f-tiles ({jj},{mm}) never queued"
                    pop_filler(1, no_pending=True)

            # seed the stream with (0, 0) scores
            for u in s_units(0, 0):
                u()

            for j in range(NJ):
                if j + 1 < NJ:
                    # q/k f-tiles for m=0 plus v(j+1) must land during j; the
                    # later m f-tiles are deferred into j+1 itself so its
                    # attention phases (Act-heavy) keep PE fed
                    fillers.extend(
                        [lambda jj=j + 1, ff=f: s1_f(jj, ff) for f in (0, 4)]
                    )
                    fillers.extend(
                        [lambda jj=j + 1, ss=s4: s1_v(jj, ss)
                         for s4 in range(4)]
                    )
                    if j + 2 < NJ:
                        fillers.append(lambda jj=j + 2: s1_load(jj))
                if j == 2:  # outproj(0) deferred to here
                    fillers.extend(
                        [lambda tt=t: outproj(0, tt) for t in range(8)]
                    )
                elif j == 3:  # outproj(1) and (2) as j=3 fillers
                    fillers.extend(
                        [lambda tt=t: outproj(1, tt) for t in range(8)]
                    )
                    fillers.extend(
                        [lambda tt=t: outproj(2, tt) for t in range(0)]
                    )
                yT = ypool.tile([128, 4, SB], BF16)
                yts[j] = yT
                for m in range(4):
                    # this j's own later q/k f-tiles, just in time
                    # (j=0 built f-tiles 1,5 upfront already)
                    mo = m + 2 if j == 0 else m + 1
                    if mo < 4:
                        fillers.insert(0, lambda jj=j, ff=mo + 4: s1_f(jj, ff))
                        fillers.insert(0, lambda jj=j, ff=mo: s1_f(jj, ff))
                    psy = py.tile([128, 4, 2, 64], F32)
                    yn = ynpool.tile([128, 4, 128], BF16)
                    # next (j, m)'s scores stream inside this A-phase; hold
                    # back the last 2 pairs so the next A-phase's first AV
                    # does not collide with this m's psy normalizes
                    if (j, m) == (NJ - 1, 3):
                        feed, post = [], []
                        tail_mode[0] = True
                    else:
                        jn, mn = (j, m + 1) if m < 3 else (j + 1, 0)
                        ensure_f(jn, mn)
                        un = s_units(jn, mn)
                        feed, post = un[:max(0, len(un) - 2)], un[-2:]
                    a_phase(j, m, epair_map[(j, m)], psy, yn, feed)
                    for u in post:
                        u()
                if j == NJ - 1:
                    # reserved chains: PE work while DVE drains the last
                    # m's normalizes, so the transpose flush doesn't stall
                    for t in range(0, 8):
                        outproj(2, t)
                        if t >= 4 and pending_t:
                            pending_t.pop(0)()
                # drain this j's fillers before moving on (fillers first:
                # the transposes' divides are still draining on DVE)
                pop_filler(len(fillers), no_pending=True)
                flush_t()
            for t in range(8):
                outproj(NJ - 1, t, tail=True)

    nc.finalize()
    return nc


def _to_bf16(a):
    import ml_dtypes
    return np.ascontiguousarray(a).astype(ml_dtypes.bfloat16)


def _prep_core_inputs(x, w_qkv, b_qkv, w_out, core):
    b = core // 2
    g = core % 2
    rows = np.concatenate([
        w_qkv[512 * g:512 * g + 512],
        w_qkv[1024 + 512 * g:1024 + 512 * g + 512],
        w_qkv[2048 + 512 * g:2048 + 512 * g + 512],
    ], axis=0)  # [1536, 1024]
    brows = np.concatenate([
        b_qkv[512 * g:512 * g + 512],
        b_qkv[1024 + 512 * g:1024 + 512 * g + 512],
        b_qkv[2048 + 512 * g:2048 + 512 * g + 512],
    ])  # [1536]

    xT = np.ascontiguousarray(
        x[b].T.reshape(DT, 128, S).transpose(1, 0, 2)
    )
    rT = rows.T.reshape(DT, 128, 12, 128)          # [d, p, ftile, fc]
    wqk = np.ascontiguousarray(rT[:, :, :8].transpose(1, 2, 0, 3))
    wvd = np.ascontiguousarray(
        rows.T.reshape(DT, 128, 1536)[:, :, 1024:].transpose(1, 0, 2)
    )
    ws = w_out[:, 512 * g:512 * g + 512]           # [do(1024), f(512)]
    wod = np.ascontiguousarray(
        ws.reshape(8, 128, 4, 128).transpose(3, 0, 2, 1))
    bqkv = np.ascontiguousarray(
        brows[:1536].reshape(12, 128).T).astype(np.float32)
    bvb = _to_bf16(brows[1024:1536].reshape(1, 512))
    tri = (np.arange(128)[:, None] <= np.arange(128)[None, :]).astype(
        np.float32)
    ident = np.eye(128, dtype=np.float32)

    return {
        "xT": _to_bf16(xT), "wqk": _to_bf16(wqk), "wvd": _to_bf16(wvd),
        "wod": _to_bf16(wod), "bqkv": bqkv, "bv": bvb,
        "tri": _to_bf16(tri), "ident": _to_bf16(ident),
    }


def kernel(x, w_qkv, b_qkv, w_out, b_out):
    global _CACHED_NC
    x = np.asarray(x, dtype=np.float32)
    w_qkv = np.asarray(w_qkv, dtype=np.float32)
    b_qkv = np.asarray(b_qkv, dtype=np.float32)
    w_out = np.asarray(w_out, dtype=np.float32)
    b_out = np.asarray(b_out, dtype=np.float32)

    if _CACHED_NC is None:
        _CACHED_NC = build_nc()
    nc = _CACHED_NC

    in_maps = [
        _prep_core_inputs(x, w_qkv, b_qkv, w_out, c) for c in range(8)
    ]
    last_err = None
    for attempt in range(5):
        try:
            res = run_bass_kernel_spmd(nc, in_maps, core_ids=list(range(8)))
            break
        except Exception as e:  # transient NRT/axon wedge: retry
            last_err = e
            import time
            time.sleep(20)
    else:
        raise last_err

    out = np.empty((B, S, D), dtype=np.float32)
    for b in range(B):
        p0 = np.asarray(res.results[2 * b]["out_part"]).astype(np.float32)
        p1_ = np.asarray(res.results[2 * b + 1]["out_part"]).astype(np.float32)
        tot = (p0 + p1_).reshape(D, S)  # [do, s]
        out[b] = tot.T + b_out[None, :]
    return out


# revision 4
# speedup vs baseline: 1.0165x; 1.0074x over previous
"""Causal self-attention (B=4, S=2048, D=1024, H=16) on 8 TRN2 NeuronCores, v2.

Sharding: core c -> batch b = c//2, head-group g = c%2 (8 heads each).

v2 design (vs baseline): all matmul operands bf16; attention-value product in
natural [q, f] layout (65 -> 64+1 rows per block via separate 1-row denominator
matmuls, halving AV tensor time); exp over 2-chunk score pairs with shifted
packing on diagonal pairs; causal triangle masked by a Pool tensor_tensor over
both triangle blocks of a pair; y transposed back to feature-major via PE
transposes; out-proj PSUM->SBUF moves on the Activation engine; stage1(j+1) and
outproj(j-1) matmuls woven between score/AV work to keep PE busy.

Layouts (per core):
  xT    [128, 8, 2048] bf16   xT[p,t,s] = x[b][s, t*128+p]
  wqk   [128, 8, 8, 128] bf16 q/k projection, f-tile-major
  wvd   [128, 8, 512] bf16    v projection
  wod   [128, 8, 4, 128] bf16 out projection (f-major partitions)
  qt/kt [128, 4, 512] bf16    feature-major q/k; partitions = 2 heads x 64
  vp    [128, 4, 512] bf16    natural v [s, f] per 128-seq chunk
  e     [128, 1024] bf16      exp(scores^T) pair tiles [k=128, q up to 2x512]
  psy   [128, 4, 2, 64] f32   natural y accum per m-pair (1 PSUM bank)
  pd    [128, 32] f32         denominators (m, half, subchunk)
  yT    [128, 4, 512] bf16    feature-major y for out-proj
"""

import numpy as np

import concourse.bacc as bacc
import concourse.mybir as mybir
from concourse.tile import TileContext
from concourse.bass_utils import run_bass_kernel_spmd

BF16 = mybir.dt.bfloat16
F32 = mybir.dt.float32
AF = mybir.ActivationFunctionType
OP = mybir.AluOpType

B, S, D = 4, 2048, 1024
H = 16
HD = 64
HL = 8          # heads per core
SB = 512        # sequence block
NJ = S // SB    # 4 s-blocks
DT = D // 128   # 8 contraction tiles

_CACHED_NC = None


def build_nc():
    nc = bacc.Bacc(None, target_bir_lowering=False)

    xT = nc.dram_tensor("xT", [128, DT, S], BF16, kind="ExternalInput")
    wqk = nc.dram_tensor("wqk", [128, 8, DT, 128], BF16, kind="ExternalInput")
    wvd = nc.dram_tensor("wvd", [128, DT, 512], BF16, kind="ExternalInput")
    wod = nc.dram_tensor("wod", [128, 8, 4, 128], BF16, kind="ExternalInput")
    bqkv = nc.dram_tensor("bqkv", [128, 12], F32, kind="ExternalInput")
    bv = nc.dram_tensor("bv", [1, 512], BF16, kind="ExternalInput")
    tri = nc.dram_tensor("tri", [128, 128], BF16, kind="ExternalInput")
    ident = nc.dram_tensor("ident", [128, 128], BF16, kind="ExternalInput")
    out = nc.dram_tensor("out_part", [8, 128, S], BF16, kind="ExternalOutput")

    with TileContext(nc) as tc:
        with (
            tc.tile_pool(name="const", bufs=1) as cpool,
            tc.tile_pool(name="big", bufs=1) as bpool,
            tc.tile_pool(name="qq", bufs=2) as qpool,
            tc.tile_pool(name="kk", bufs=4) as kpool,
            tc.tile_pool(name="vv", bufs=4) as vpool,
            tc.tile_pool(name="xb", bufs=2) as xpool,
            tc.tile_pool(name="ee", bufs=36) as epool,
            tc.tile_pool(name="yn", bufs=4) as ynpool,
            tc.tile_pool(name="rd", bufs=3) as rdpool,
            tc.tile_pool(name="yt", bufs=3) as ypool,
            tc.tile_pool(name="ob", bufs=5) as opool,
            tc.tile_pool(name="p1", bufs=2, space="PSUM") as p1,
            tc.tile_pool(name="pscore", bufs=2, space="PSUM") as pscore,
            tc.tile_pool(name="py", bufs=1, space="PSUM") as py,
            tc.tile_pool(name="paux", bufs=1, space="PSUM") as pauxpool,
        ):
            # ---- constants (tiles now; DMAs emitted after the first x/w
            # loads so the hot path owns the DMA queues at startup) ----
            tri_t = cpool.tile([128, 128], BF16)
            ident_t = cpool.tile([128, 128], BF16)
            bqkv_t = cpool.tile([128, 12], F32)
            bv_t = cpool.tile([1, 512], BF16)
            bvb = cpool.tile([128, 512], F32)
            ones = cpool.tile([128, 1], BF16)
            nc.vector.memset(ones[:], 1.0)
            ones_row = cpool.tile([1, 128], BF16)
            nc.vector.memset(ones_row[:], 1.0)

            wv = bpool.tile([128, DT, 512], BF16)
            wod_t = bpool.tile([128, 8, 4, 128], BF16)
            wqk_t = bpool.tile([128, 8, DT, 128], BF16)

            xblk = {}
            qk_blk = {}
            v_blk = {}
            yts = {}

            # one PSUM bank shared by denominators (cols 0:32 f32) and
            # rotating transpose-output regions (3 x [128,128] bf16)
            aux = pauxpool.tile([128, 512], F32)
            pt_ctr = [0]

            def pt_region():
                r = pt_ctr[0] % 3
                pt_ctr[0] += 1
                lo = 32 + r * 64
                return aux[:, lo:lo + 64].bitcast(BF16)

            # ---- stage 1 pieces (emitted interleaved with attention) ----
            def s1_load(j, split=False):
                xb = xpool.tile([128, DT, SB], BF16)
                for d in range(DT):  # split so first matmul starts early
                    eng = nc.scalar if (split and d % 2) else nc.sync
                    eng.dma_start(xb[:, d, :], xT[:, d, j * SB:(j + 1) * SB])
                qt = qpool.tile([128, 4, SB], BF16)
                kt = kpool.tile([128, 4, SB], BF16)
                qk_blk[j] = kt
                xblk[j] = (xb, qt, kt)

            def s1_f(j, f):
                xb, qt, kt = xblk[j]
                ps = p1.tile([128, SB], F32, tag="ps")
                for d in range(DT):
                    nc.tensor.matmul(
                        ps[:], wqk_t[:, f, d, :], xb[:, d, :],
                        start=(d == 0), stop=(d == DT - 1),
                    )
                dst = qt[:, f, :] if f < 4 else kt[:, f - 4, :]
                nc.vector.tensor_scalar(
                    dst, ps[:], bqkv_t[:, f:f + 1], None, OP.add
                )
                emitted_f.add((j, f))

            def s1_v(j, s4):
                xb = xblk[j][0]
                if s4 == 0:
                    vp = vpool.tile([128, 4, 512], BF16)
                    v_blk[j] = vp
                vp = v_blk[j]
                ps = p1.tile([128, 512], F32, tag="ps")
                for d in range(DT):
                    nc.tensor.matmul(
                        ps[:], xb[:, d, s4 * 128:(s4 + 1) * 128], wv[:, d, :],
                        start=(d == 0), stop=(d == DT - 1),
                    )
                nc.vector.tensor_tensor(vp[:, s4, :], ps[:], bvb[:], OP.add)

            def outproj(j, t, tail=False):
                yT = yts[j]
                ps = p1.tile([128, SB], F32, tag="ps")
                for ff in range(4):
                    nc.tensor.matmul(
                        ps[:], wod_t[:, t, ff, :], yT[:, ff, :],
                        start=(ff == 0), stop=(ff == 3),
                    )
                ob = opool.tile([128, SB], BF16)
                nc.vector.tensor_copy(ob[:], ps[:])
                eng = nc.scalar if (j == NJ - 1 and t % 2) else nc.sync
                eng.dma_start(out[t, :, j * SB:(j + 1) * SB], ob[:])

            # ---- filler management ----
            fillers = []
            pending_t = []  # deferred transposes (keep newest 1 in flight)
            epair_map = {}
            emitted_f = set()
            tail_mode = [False]

            def pop_filler(k=1, no_pending=False):
                for _ in range(k):
                    if len(pending_t) > 1 and not no_pending:
                        pending_t.pop(0)()
                    elif fillers:
                        fillers.pop(0)()

            def flush_t():
                while pending_t:
                    pending_t.pop(0)()

            # ---- attention per (j, m): score units + A-phase ----
            def s_unit(j, m, p, e_pairs):
                """Scores + exp for one k-chunk pair of head-pair m, block j.

                e_pairs[(half, p)] = (e_tile, [col offset of chunk 2p, 2p+1]).
                Diagonal chunks (global chunk index i >= 4j) are written
                column-shifted so each pair's valid region is contiguous
                from its slot start; exp then covers exactly the valid span
                (plus any gap between the two chunks' slots, never read).
                """
                qk = xblk[j][1]
                diag = 2 * p >= 4 * j
                if True:
                    for half in range(2):
                        b0 = half * 64
                        ps = pscore.tile([128, 1024], F32)
                        e = epool.tile([128, 1024], BF16)
                        offs = []
                        base = 0
                        tri_offs = []
                        for q in range(2):
                            i = 2 * p + q
                            ib, il = i // 4, i % 4
                            c0 = max(0, i * 128 - j * SB)
                            n = SB - c0
                            # shifted packing: valid q-range [c0, 512) lands
                            # at cols [base, base + n); slot must not cross a
                            # 512-col PSUM bank boundary
                            if q == 1 and base + n > 512:
                                base = 512
                            ks = qk_blk[ib][b0:b0 + 64, m,
                                            il * 128:il * 128 + 128]
                            qs = qk[b0:b0 + 64, m, c0:SB]
                            nc.tensor.matmul(
                                ps[:, base:base + n], ks, qs,
                                start=True, stop=True,
                            )
                            offs.append(base)
                            if diag:
                                tri_offs.append(base)
                            base += n
                        # one exp over the union [0, end of odd slot)
                        nc.scalar.activation(
                            e[:, :base], ps[:, :base], AF.Exp, scale=0.125
                        )
                        for to in tri_offs:  # mask causal triangles
                            nc.gpsimd.tensor_tensor(
                                e[:, to:to + 128], e[:, to:to + 128],
                                tri_t[:], OP.mult,
                            )
                        e_pairs[(half, p)] = (e, offs)
                    if p % 2 == 0 or j < 2:
                        pop_filler(1)

            def s_units(j, m):
                """Per-pair emission thunks, diagonal pairs first: the
                A-phase consumes every pair in its very first s-chain, so the
                last-produced pair must not be the one with the longest
                exp+mask latency."""
                e_pairs = {}
                epair_map[(j, m)] = e_pairs
                n_sk = 4 * (j + 1)
                order = list(range(2 * j, n_sk // 2)) + list(range(2 * j))
                return [
                    (lambda pp=p: s_unit(j, m, pp, e_pairs)) for p in order
                ]

            def a_phase(j, m, e_pairs, psy, yn, feed=()):
                """AV in natural [q, f] layout + denominators + normalize +
                transpose into yT[j][:, m, :]."""
                yT = yts[j]
                for s in range(4):
                    n_i = 4 * j + s + 1
                    for half in range(2):
                        h = 2 * m + half
                        for i in range(n_i):
                            ib, il = i // 4, i % 4
                            c0 = max(0, i * 128 - j * SB)
                            e, offs = e_pairs[(half, i // 2)]
                            col = offs[i % 2] + s * 128 - c0
                            el = e[:, col:col + 128]
                            vs = v_blk[ib][:, il, h * 64:h * 64 + 64]
                            nc.tensor.matmul(
                                psy[:, s, half, :], el, vs,
                                start=(i == 0), stop=(i == n_i - 1),
                            )
                            dcol = (m * 2 + half) * 4 + s
                            nc.tensor.matmul(
                                aux[:, dcol:dcol + 1], el, ones[:],
                                start=(i == 0), stop=(i == n_i - 1),
                            )
                    # feed the next (j, m)'s score pairs between AV chains
                    # so the Activation engine never drains at m boundaries
                    for u in feed[2 * s:2 * s + 2]:
                        u()
                    pop_filler(1)
                # normalize after all AV chains: hazards are tracked per
                # tile, so touching psy/aux between chains would stall the
                # next chain's accumulation. DVE has no divide: recip + mult.
                rd = rdpool.tile([128, 8], F32)
                nc.vector.reciprocal(rd[:], aux[:, m * 8:m * 8 + 8])
                for s in range(4):
                    for half in range(2):
                        dcol = half * 4 + s
                        nc.vector.tensor_scalar(
                            yn[:, s, half * 64:half * 64 + 64],
                            psy[:, s, half, :],
                            rd[:, dcol:dcol + 1], None, OP.mult,
                        )

                    def transp(mm=m, ss=s, yy=yn):
                        ptb = pt_region()
                        nc.tensor.transpose(ptb, yy[:, ss, :], ident_t[:])
                        nc.vector.tensor_copy(
                            yT[:, mm, ss * 128:ss * 128 + 128], ptb)

                    # defer: this s-group's divide is still in flight on DVE;
                    # running the transpose now would stall PE
                    pending_t.append(transp)

            # ---- main pipeline ----
            nc.scalar.dma_start(wqk_t[:, 0], wqk[:, 0])
            s1_load(0, split=True)
            nc.gpsimd.dma_start(bqkv_t[:], bqkv[:])
            nc.gpsimd.dma_start(bv_t[:], bv[:])
            nc.sync.dma_start(wqk_t[:, 4], wqk[:, 4])
            s1_f(0, 0)
            nc.sync.dma_start(wv[:], wvd[:])
            nc.gpsimd.dma_start(tri_t[:], tri[:])
            nc.gpsimd.dma_start(ident_t[:], ident[:])
            for f in (1, 2, 3, 5, 6, 7):
                nc.sync.dma_start(wqk_t[:, f], wqk[:, f])
            s1_f(0, 4)
            # broadcast bv to all partitions via a rank-1 matmul (cheaper
            # than a gpsimd partition_broadcast; placed here so the wait on
            # the bv DMA is hidden behind the first two qk chains)
            ps_b = p1.tile([128, 512], F32, tag="ps")
            nc.tensor.matmul(ps_b[:], ones_row[:], bv_t[:], start=True,
                             stop=True)
            nc.vector.tensor_copy(bvb[:], ps_b[:])
            s1_f(0, 1)
            s1_f(0, 5)
            for s4 in range(4):
                s1_v(0, s4)
            s1_load(1)
            nc.sync.dma_start(wod_t[:], wod[:])

            def ensure_f(jj, mm):
                """Pop fillers until q/k f-tiles (mm, mm+4) of block jj have
                been emitted (score units for (jj, mm) read them)."""
                guard = len(fillers) + 4
                while not ({(jj, mm), (jj, mm + 4)} <= emitted_f):
                    guard -= 1
                    assert guard > 0, f"f-tiles ({jj},{mm}) never queued"
                    pop_filler(1, no_pending=True)

            # seed the stream with (0, 0) scores
            for u in s_units(0, 0):
                u()

            for j in range(NJ):
                if j + 1 < NJ:
                    # q/k f-tiles for m=0 plus v(j+1) must land during j; the
                    # later m f-tiles are deferred into j+1 itself so its
                    # attention phases (Act-heavy) keep PE fed
                    fillers.extend(
                        [lambda jj=j + 1, ff=f: s1_f(jj, ff) for f in (0, 4)]
                    )
                    fillers.extend(
                        [lambda jj=j + 1, ss=s4: s1_v(jj, ss)
                         for s4 in range(4)]
                    )
                    if j + 2 < NJ:
                        fillers.append(lambda jj=j + 2: s1_load(jj))
                if j == 2:  # outproj(0) deferred to here
                    fillers.extend(
                        [lambda tt=t: outproj(0, tt) for t in range(8)]
                    )
                elif j == 3:  # outproj(1) and (2) as j=3 fillers
                    fillers.extend(
                        [lambda tt=t: outproj(1, tt) for t in range(8)]
                    )
                    fillers.extend(
                        [lambda tt=t: outproj(2, tt) for t in range(0)]
                    )
                yT = ypool.tile([128, 4, SB], BF16)
                yts[j] = yT
                for m in range(4):
                    # this j's own later q/k f-tiles, just in time
                    # (j=0 built f-tiles 1,5 upfront already)
                    mo = m + 2 if j == 0 else m + 1
                    if mo < 4:
                        fillers.insert(0, lambda jj=j, ff=mo + 4: s1_f(jj, ff))
                        fillers.insert(0, lambda jj=j, ff=mo: s1_f(jj, ff))
                    psy = py.tile([128, 4, 2, 64], F32)
                    yn = ynpool.tile([128, 4, 128], BF16)
                    # next (j, m)'s scores stream inside this A-phase; hold
                    # back the last 2 pairs so the next A-phase's first AV
                    # does not collide with this m's psy normalizes
                    if (j, m) == (NJ - 1, 3):
                        feed, post = [], []
                        tail_mode[0] = True
                    else:
                        jn, mn = (j, m + 1) if m < 3 else (j + 1, 0)
                        ensure_f(jn, mn)
                        un = s_units(jn, mn)
                        feed, post = un[:max(0, len(un) - 2)], un[-2:]
                    a_phase(j, m, epair_map[(j, m)], psy, yn, feed)
                    for u in post:
                        u()
                if j == NJ - 1:
                    # reserved chains: PE work while DVE drains the last
                    # m's normalizes, so the transpose flush doesn't stall
                    for t in range(0, 8):
                        outproj(2, t)
                        if t >= 4 and pending_t:
                            pending_t.pop(0)()
                # drain this j's fillers before moving on (fillers first:
                # the transposes' divides are still draining on DVE)
                pop_filler(len(fillers), no_pending=True)
                flush_t()
            for t in range(8):
                outproj(NJ - 1, t, tail=True)

    nc.finalize()
    return nc


def _to_bf16(a):
    import ml_dtypes
    return np.ascontiguousarray(a).astype(ml_dtypes.bfloat16)


def _prep_core_inputs(x, w_qkv, b_qkv, w_out, core):
    b = core // 2
    g = core % 2
    rows = np.concatenate([
        w_qkv[512 * g:512 * g + 512],
        w_qkv[1024 + 512 * g:1024 + 512 * g + 512],
        w_qkv[2048 + 512 * g:2048 + 512 * g + 512],
    ], axis=0)  # [1536, 1024]
    brows = np.concatenate([
        b_qkv[512 * g:512 * g + 512],
        b_qkv[1024 + 512 * g:1024 + 512 * g + 512],
        b_qkv[2048 + 512 * g:2048 + 512 * g + 512],
    ])  # [1536]

    xT = np.ascontiguousarray(
        x[b].T.reshape(DT, 128, S).transpose(1, 0, 2)
    )
    rT = rows.T.reshape(DT, 128, 12, 128)          # [d, p, ftile, fc]
    wqk = np.ascontiguousarray(rT[:, :, :8].transpose(1, 2, 0, 3))
    wvd = np.ascontiguousarray(
        rows.T.reshape(DT, 128, 1536)[:, :, 1024:].transpose(1, 0, 2)
    )
    ws = w_out[:, 512 * g:512 * g + 512]           # [do(1024), f(512)]
    wod = np.ascontiguousarray(
        ws.reshape(8, 128, 4, 128).transpose(3, 0, 2, 1))
    bqkv = np.ascontiguousarray(
        brows[:1536].reshape(12, 128).T).astype(np.float32)
    bvb = _to_bf16(brows[1024:1536].reshape(1, 512))
    tri = (np.arange(128)[:, None] <= np.arange(128)[None, :]).astype(
        np.float32)
    ident = np.eye(128, dtype=np.float32)

    return {
        "xT": _to_bf16(xT), "wqk": _to_bf16(wqk), "wvd": _to_bf16(wvd),
        "wod": _to_bf16(wod), "bqkv": bqkv, "bv": bvb,
        "tri": _to_bf16(tri), "ident": _to_bf16(ident),
    }


def kernel(x, w_qkv, b_qkv, w_out, b_out):
    global _CACHED_NC
    x = np.asarray(x, dtype=np.float32)
    w_qkv = np.asarray(w_qkv, dtype=np.float32)
    b_qkv = np.asarray(b_qkv, dtype=np.float32)
    w_out = np.asarray(w_out, dtype=np.float32)
    b_out = np.asarray(b_out, dtype=np.float32)

    if _CACHED_NC is None:
        _CACHED_NC = build_nc()
    nc = _CACHED_NC

    in_maps = [
        _prep_core_inputs(x, w_qkv, b_qkv, w_out, c) for c in range(8)
    ]
    last_err = None
    for attempt in range(5):
        try:
            res = run_bass_kernel_spmd(nc, in_maps, core_ids=list(range(8)))
            break
        except Exception as e:  # transient NRT/axon wedge: retry
            last_err = e
            import time
            time.sleep(20)
    else:
        raise last_err

    out = np.empty((B, S, D), dtype=np.float32)
    for b in range(B):
        p0 = np.asarray(res.results[2 * b]["out_part"]).astype(np.float32)
        p1_ = np.asarray(res.results[2 * b + 1]["out_part"]).astype(np.float32)
        tot = (p0 + p1_).reshape(D, S)  # [do, s]
        out[b] = tot.T + b_out[None, :]
    return out


# revision 5
# speedup vs baseline: 1.0174x; 1.0009x over previous
"""Causal self-attention (B=4, S=2048, D=1024, H=16) on 8 TRN2 NeuronCores, v2.

Sharding: core c -> batch b = c//2, head-group g = c%2 (8 heads each).

v2 design (vs baseline): all matmul operands bf16; attention-value product in
natural [q, f] layout (65 -> 64+1 rows per block via separate 1-row denominator
matmuls, halving AV tensor time); exp over 2-chunk score pairs with shifted
packing on diagonal pairs; causal triangle masked by a Pool tensor_tensor over
both triangle blocks of a pair; y transposed back to feature-major via PE
transposes; out-proj PSUM->SBUF moves on the Activation engine; stage1(j+1) and
outproj(j-1) matmuls woven between score/AV work to keep PE busy.

Layouts (per core):
  xT    [128, 8, 2048] bf16   xT[p,t,s] = x[b][s, t*128+p]
  wqk   [128, 8, 8, 128] bf16 q/k projection, f-tile-major
  wvd   [128, 8, 512] bf16    v projection
  wod   [128, 8, 4, 128] bf16 out projection (f-major partitions)
  qt/kt [128, 4, 512] bf16    feature-major q/k; partitions = 2 heads x 64
  vp    [128, 4, 512] bf16    natural v [s, f] per 128-seq chunk
  e     [128, 1024] bf16      exp(scores^T) pair tiles [k=128, q up to 2x512]
  psy   [128, 4, 2, 64] f32   natural y accum per m-pair (1 PSUM bank)
  pd    [128, 32] f32         denominators (m, half, subchunk)
  yT    [128, 4, 512] bf16    feature-major y for out-proj
"""

import numpy as np

import concourse.bacc as bacc
import concourse.mybir as mybir
from concourse.tile import TileContext
from concourse.bass_utils import run_bass_kernel_spmd

BF16 = mybir.dt.bfloat16
F32 = mybir.dt.float32
AF = mybir.ActivationFunctionType
OP = mybir.AluOpType

B, S, D = 4, 2048, 1024
H = 16
HD = 64
HL = 8          # heads per core
SB = 512        # sequence block
NJ = S // SB    # 4 s-blocks
DT = D // 128   # 8 contraction tiles

_CACHED_NC = None


def build_nc():
    nc = bacc.Bacc(None, target_bir_lowering=False)

    xT = nc.dram_tensor("xT", [128, DT, S], BF16, kind="ExternalInput")
    wqk = nc.dram_tensor("wqk", [128, 8, DT, 128], BF16, kind="ExternalInput")
    wvd = nc.dram_tensor("wvd", [128, DT, 512], BF16, kind="ExternalInput")
    wod = nc.dram_tensor("wod", [128, 8, 4, 128], BF16, kind="ExternalInput")
    bqkv = nc.dram_tensor("bqkv", [128, 12], F32, kind="ExternalInput")
    bv = nc.dram_tensor("bv", [1, 512], BF16, kind="ExternalInput")
    tri = nc.dram_tensor("tri", [128, 128], BF16, kind="ExternalInput")
    ident = nc.dram_tensor("ident", [128, 128], BF16, kind="ExternalInput")
    out = nc.dram_tensor("out_part", [8, 128, S], BF16, kind="ExternalOutput")

    with TileContext(nc) as tc:
        with (
            tc.tile_pool(name="const", bufs=1) as cpool,
            tc.tile_pool(name="big", bufs=1) as bpool,
            tc.tile_pool(name="qq", bufs=2) as qpool,
            tc.tile_pool(name="kk", bufs=4) as kpool,
            tc.tile_pool(name="vv", bufs=4) as vpool,
            tc.tile_pool(name="xb", bufs=2) as xpool,
            tc.tile_pool(name="ee", bufs=40) as epool,
            tc.tile_pool(name="yn", bufs=4) as ynpool,
            tc.tile_pool(name="rd", bufs=3) as rdpool,
            tc.tile_pool(name="yt", bufs=3) as ypool,
            tc.tile_pool(name="ob", bufs=5) as opool,
            tc.tile_pool(name="p1", bufs=2, space="PSUM") as p1,
            tc.tile_pool(name="pscore", bufs=2, space="PSUM") as pscore,
            tc.tile_pool(name="py", bufs=1, space="PSUM") as py,
            tc.tile_pool(name="paux", bufs=1, space="PSUM") as pauxpool,
        ):
            # ---- constants (tiles now; DMAs emitted after the first x/w
            # loads so the hot path owns the DMA queues at startup) ----
            tri_t = cpool.tile([128, 128], BF16)
            ident_t = cpool.tile([128, 128], BF16)
            bqkv_t = cpool.tile([128, 12], F32)
            bv_t = cpool.tile([1, 512], BF16)
            bvb = cpool.tile([128, 512], F32)
            ones = cpool.tile([128, 1], BF16)
            nc.vector.memset(ones[:], 1.0)
            ones_row = cpool.tile([1, 128], BF16)
            nc.vector.memset(ones_row[:], 1.0)

            wv = bpool.tile([128, DT, 512], BF16)
            wod_t = bpool.tile([128, 8, 4, 128], BF16)
            wqk_t = bpool.tile([128, 8, DT, 128], BF16)

            xblk = {}
            qk_blk = {}
            v_blk = {}
            yts = {}

            # one PSUM bank shared by denominators (cols 0:32 f32) and
            # rotating transpose-output regions (3 x [128,128] bf16)
            aux = pauxpool.tile([128, 512], F32)
            pt_ctr = [0]

            def pt_region():
                r = pt_ctr[0] % 3
                pt_ctr[0] += 1
                lo = 32 + r * 64
                return aux[:, lo:lo + 64].bitcast(BF16)

            # ---- stage 1 pieces (emitted interleaved with attention) ----
            def s1_load(j, split=False):
                xb = xpool.tile([128, DT, SB], BF16)
                for d in range(DT):  # split so first matmul starts early
                    eng = nc.scalar if (split and d % 2) else nc.sync
                    eng.dma_start(xb[:, d, :], xT[:, d, j * SB:(j + 1) * SB])
                qt = qpool.tile([128, 4, SB], BF16)
                kt = kpool.tile([128, 4, SB], BF16)
                qk_blk[j] = kt
                xblk[j] = (xb, qt, kt)

            def s1_f(j, f):
                xb, qt, kt = xblk[j]
                ps = p1.tile([128, SB], F32, tag="ps")
                for d in range(DT):
                    nc.tensor.matmul(
                        ps[:], wqk_t[:, f, d, :], xb[:, d, :],
                        start=(d == 0), stop=(d == DT - 1),
                    )
                dst = qt[:, f, :] if f < 4 else kt[:, f - 4, :]
                nc.vector.tensor_scalar(
                    dst, ps[:], bqkv_t[:, f:f + 1], None, OP.add
                )
                emitted_f.add((j, f))

            def s1_v(j, s4):
                xb = xblk[j][0]
                if s4 == 0:
                    vp = vpool.tile([128, 4, 512], BF16)
                    v_blk[j] = vp
                vp = v_blk[j]
                ps = p1.tile([128, 512], F32, tag="ps")
                for d in range(DT):
                    nc.tensor.matmul(
                        ps[:], xb[:, d, s4 * 128:(s4 + 1) * 128], wv[:, d, :],
                        start=(d == 0), stop=(d == DT - 1),
                    )
                nc.vector.tensor_tensor(vp[:, s4, :], ps[:], bvb[:], OP.add)

            def outproj(j, t, tail=False):
                yT = yts[j]
                ps = p1.tile([128, SB], F32, tag="ps")
                for ff in range(4):
                    nc.tensor.matmul(
                        ps[:], wod_t[:, t, ff, :], yT[:, ff, :],
                        start=(ff == 0), stop=(ff == 3),
                    )
                ob = opool.tile([128, SB], BF16)
                nc.vector.tensor_copy(ob[:], ps[:])
                eng = nc.scalar if (j == NJ - 1 and t % 2) else nc.sync
                eng.dma_start(out[t, :, j * SB:(j + 1) * SB], ob[:])

            # ---- filler management ----
            fillers = []
            pending_t = []  # deferred transposes (keep newest 1 in flight)
            epair_map = {}
            emitted_f = set()
            tail_mode = [False]

            def pop_filler(k=1, no_pending=False):
                for _ in range(k):
                    if len(pending_t) > 1 and not no_pending:
                        pending_t.pop(0)()
                    elif fillers:
                        fillers.pop(0)()

            def flush_t():
                while pending_t:
                    pending_t.pop(0)()

            # ---- attention per (j, m): score units + A-phase ----
            def s_unit(j, m, p, e_pairs):
                """Scores + exp for one k-chunk pair of head-pair m, block j.

                e_pairs[(half, p)] = (e_tile, [col offset of chunk 2p, 2p+1]).
                Diagonal chunks (global chunk index i >= 4j) are written
                column-shifted so each pair's valid region is contiguous
                from its slot start; exp then covers exactly the valid span
                (plus any gap between the two chunks' slots, never read).
                """
                qk = xblk[j][1]
                diag = 2 * p >= 4 * j
                if True:
                    for half in range(2):
                        b0 = half * 64
                        ps = pscore.tile([128, 1024], F32)
                        e = epool.tile([128, 1024], BF16)
                        offs = []
                        base = 0
                        tri_offs = []
                        for q in range(2):
                            i = 2 * p + q
                            ib, il = i // 4, i % 4
                            c0 = max(0, i * 128 - j * SB)
                            n = SB - c0
                            # shifted packing: valid q-range [c0, 512) lands
                            # at cols [base, base + n); slot must not cross a
                            # 512-col PSUM bank boundary
                            if q == 1 and base + n > 512:
                                base = 512
                            ks = qk_blk[ib][b0:b0 + 64, m,
                                            il * 128:il * 128 + 128]
                            qs = qk[b0:b0 + 64, m, c0:SB]
                            nc.tensor.matmul(
                                ps[:, base:base + n], ks, qs,
                                start=True, stop=True,
                            )
                            offs.append(base)
                            if diag:
                                tri_offs.append(base)
                            base += n
                        # one exp over the union [0, end of odd slot)
                        nc.scalar.activation(
                            e[:, :base], ps[:, :base], AF.Exp, scale=0.125
                        )
                        for to in tri_offs:  # mask causal triangles
                            nc.gpsimd.tensor_tensor(
                                e[:, to:to + 128], e[:, to:to + 128],
                                tri_t[:], OP.mult,
                            )
                        e_pairs[(half, p)] = (e, offs)
                    if p % 2 == 0 or j < 2:
                        pop_filler(1)

            def s_units(j, m):
                """Per-pair emission thunks, diagonal pairs first: the
                A-phase consumes every pair in its very first s-chain, so the
                last-produced pair must not be the one with the longest
                exp+mask latency."""
                e_pairs = {}
                epair_map[(j, m)] = e_pairs
                n_sk = 4 * (j + 1)
                order = list(range(2 * j, n_sk // 2)) + list(range(2 * j))
                return [
                    (lambda pp=p: s_unit(j, m, pp, e_pairs)) for p in order
                ]

            def a_phase(j, m, e_pairs, psy, yn, feed=()):
                """AV in natural [q, f] layout + denominators + normalize +
                transpose into yT[j][:, m, :]."""
                yT = yts[j]
                for s in range(4):
                    n_i = 4 * j + s + 1
                    for half in range(2):
                        h = 2 * m + half
                        for i in range(n_i):
                            ib, il = i // 4, i % 4
                            c0 = max(0, i * 128 - j * SB)
                            e, offs = e_pairs[(half, i // 2)]
                            col = offs[i % 2] + s * 128 - c0
                            el = e[:, col:col + 128]
                            vs = v_blk[ib][:, il, h * 64:h * 64 + 64]
                            nc.tensor.matmul(
                                psy[:, s, half, :], el, vs,
                                start=(i == 0), stop=(i == n_i - 1),
                            )
                            dcol = (m * 2 + half) * 4 + s
                            nc.tensor.matmul(
                                aux[:, dcol:dcol + 1], el, ones[:],
                                start=(i == 0), stop=(i == n_i - 1),
                            )
                    # feed the next (j, m)'s score pairs between AV chains
                    # so the Activation engine never drains at m boundaries
                    for u in feed[4 * s:4 * s + 4]:
                        u()
                    pop_filler(1)
                # normalize after all AV chains: hazards are tracked per
                # tile, so touching psy/aux between chains would stall the
                # next chain's accumulation. DVE has no divide: recip + mult.
                rd = rdpool.tile([128, 8], F32)
                nc.vector.reciprocal(rd[:], aux[:, m * 8:m * 8 + 8])
                for s in range(4):
                    for half in range(2):
                        dcol = half * 4 + s
                        nc.vector.tensor_scalar(
                            yn[:, s, half * 64:half * 64 + 64],
                            psy[:, s, half, :],
                            rd[:, dcol:dcol + 1], None, OP.mult,
                        )

                    def transp(mm=m, ss=s, yy=yn):
                        ptb = pt_region()
                        nc.tensor.transpose(ptb, yy[:, ss, :], ident_t[:])
                        nc.vector.tensor_copy(
                            yT[:, mm, ss * 128:ss * 128 + 128], ptb)

                    # defer: this s-group's divide is still in flight on DVE;
                    # running the transpose now would stall PE
                    pending_t.append(transp)

            # ---- main pipeline ----
            nc.scalar.dma_start(wqk_t[:, 0], wqk[:, 0])
            s1_load(0, split=True)
            nc.gpsimd.dma_start(bqkv_t[:], bqkv[:])
            nc.gpsimd.dma_start(bv_t[:], bv[:])
            nc.sync.dma_start(wqk_t[:, 4], wqk[:, 4])
            s1_f(0, 0)
            nc.sync.dma_start(wv[:], wvd[:])
            nc.gpsimd.dma_start(tri_t[:], tri[:])
            nc.gpsimd.dma_start(ident_t[:], ident[:])
            for f in (1, 2, 3, 5, 6, 7):
                nc.sync.dma_start(wqk_t[:, f], wqk[:, f])
            s1_f(0, 4)
            # broadcast bv to all partitions via a rank-1 matmul (cheaper
            # than a gpsimd partition_broadcast; placed here so the wait on
            # the bv DMA is hidden behind the first two qk chains)
            ps_b = p1.tile([128, 512], F32, tag="ps")
            nc.tensor.matmul(ps_b[:], ones_row[:], bv_t[:], start=True,
                             stop=True)
            nc.vector.tensor_copy(bvb[:], ps_b[:])
            s1_f(0, 1)
            s1_f(0, 5)
            for s4 in range(4):
                s1_v(0, s4)
            s1_load(1)
            nc.sync.dma_start(wod_t[:], wod[:])

            def ensure_f(jj, mm):
                """Pop fillers until q/k f-tiles (mm, mm+4) of block jj have
                been emitted (score units for (jj, mm) read them)."""
                guard = len(fillers) + 4
                while not ({(jj, mm), (jj, mm + 4)} <= emitted_f):
                    guard -= 1
                    assert guard > 0, f"f-tiles ({jj},{mm}) never queued"
                    pop_filler(1, no_pending=True)

            # seed the stream with (0, 0) scores
            for u in s_units(0, 0):
                u()

            for j in range(NJ):
                if j + 1 < NJ:
                    # q/k f-tiles for m=0 plus v(j+1) must land during j; the
                    # later m f-tiles are deferred into j+1 itself so its
                    # attention phases (Act-heavy) keep PE fed
                    fillers.extend(
                        [lambda jj=j + 1, ff=f: s1_f(jj, ff) for f in (0, 4)]
                    )
                    fillers.extend(
                        [lambda jj=j + 1, ss=s4: s1_v(jj, ss)
                         for s4 in range(4)]
                    )
                    if j + 2 < NJ:
                        fillers.append(lambda jj=j + 2: s1_load(jj))
                if j == 2:  # outproj(0) deferred to here
                    fillers.extend(
                        [lambda tt=t: outproj(0, tt) for t in range(8)]
                    )
                elif j == 3:  # outproj(1) and (2) as j=3 fillers
                    fillers.extend(
                        [lambda tt=t: outproj(1, tt) for t in range(8)]
                    )
                    fillers.extend(
                        [lambda tt=t: outproj(2, tt) for t in range(0)]
                    )
                yT = ypool.tile([128, 4, SB], BF16)
                yts[j] = yT
                for m in range(4):
                    # this j's own later q/k f-tiles, just in time
                    # (j=0 built f-tiles 1,5 upfront already)
                    mo = m + 2 if j == 0 else m + 1
                    if mo < 4:
                        fillers.insert(0, lambda jj=j, ff=mo + 4: s1_f(jj, ff))
                        fillers.insert(0, lambda jj=j, ff=mo: s1_f(jj, ff))
                    psy = py.tile([128, 4, 2, 64], F32)
                    yn = ynpool.tile([128, 4, 128], BF16)
                    # next (j, m)'s scores stream inside this A-phase; hold
                    # back the last 2 pairs so the next A-phase's first AV
                    # does not collide with this m's psy normalizes
                    if (j, m) == (NJ - 1, 3):
                        feed, post = [], []
                        tail_mode[0] = True
                    else:
                        jn, mn = (j, m + 1) if m < 3 else (j + 1, 0)
                        ensure_f(jn, mn)
                        un = s_units(jn, mn)
                        feed, post = un[:max(0, len(un) - 2)], un[-2:]
                    a_phase(j, m, epair_map[(j, m)], psy, yn, feed)
                    for u in post:
                        u()
                if j == NJ - 1:
                    # reserved chains: PE work while DVE drains the last
                    # m's normalizes, so the transpose flush doesn't stall
                    for t in range(0, 8):
                        outproj(2, t)
                        if t >= 4 and pending_t:
                            pending_t.pop(0)()
                # drain this j's fillers before moving on (fillers first:
                # the transposes' divides are still draining on DVE)
                pop_filler(len(fillers), no_pending=True)
                flush_t()
            for t in range(8):
                outproj(NJ - 1, t, tail=True)

    nc.finalize()
    return nc


def _to_bf16(a):
    import ml_dtypes
    return np.ascontiguousarray(a).astype(ml_dtypes.bfloat16)


def _prep_core_inputs(x, w_qkv, b_qkv, w_out, core):
    b = core // 2
    g = core % 2
    rows = np.concatenate([
        w_qkv[512 * g:512 * g + 512],
        w_qkv[1024 + 512 * g:1024 + 512 * g + 512],
        w_qkv[2048 + 512 * g:2048 + 512 * g + 512],
    ], axis=0)  # [1536, 1024]
    brows = np.concatenate([
        b_qkv[512 * g:512 * g + 512],
        b_qkv[1024 + 512 * g:1024 + 512 * g + 512],
        b_qkv[2048 + 512 * g:2048 + 512 * g + 512],
    ])  # [1536]

    xT = np.ascontiguousarray(
        x[b].T.reshape(DT, 128, S).transpose(1, 0, 2)
    )
    rT = rows.T.reshape(DT, 128, 12, 128)          # [d, p, ftile, fc]
    wqk = np.ascontiguousarray(rT[:, :, :8].transpose(1, 2, 0, 3))
    wvd = np.ascontiguousarray(
        rows.T.reshape(DT, 128, 1536)[:, :, 1024:].transpose(1, 0, 2)
    )
    ws = w_out[:, 512 * g:512 * g + 512]           # [do(1024), f(512)]
    wod = np.ascontiguousarray(
        ws.reshape(8, 128, 4, 128).transpose(3, 0, 2, 1))
    bqkv = np.ascontiguousarray(
        brows[:1536].reshape(12, 128).T).astype(np.float32)
    bvb = _to_bf16(brows[1024:1536].reshape(1, 512))
    tri = (np.arange(128)[:, None] <= np.arange(128)[None, :]).astype(
        np.float32)
    ident = np.eye(128, dtype=np.float32)

    return {
        "xT": _to_bf16(xT), "wqk": _to_bf16(wqk), "wvd": _to_bf16(wvd),
        "wod": _to_bf16(wod), "bqkv": bqkv, "bv": bvb,
        "tri": _to_bf16(tri), "ident": _to_bf16(ident),
    }


def kernel(x, w_qkv, b_qkv, w_out, b_out):
    global _CACHED_NC
    x = np.asarray(x, dtype=np.float32)
    w_qkv = np.asarray(w_qkv, dtype=np.float32)
    b_qkv = np.asarray(b_qkv, dtype=np.float32)
    w_out = np.asarray(w_out, dtype=np.float32)
    b_out = np.asarray(b_out, dtype=np.float32)

    if _CACHED_NC is None:
        _CACHED_NC = build_nc()
    nc = _CACHED_NC

    in_maps = [
        _prep_core_inputs(x, w_qkv, b_qkv, w_out, c) for c in range(8)
    ]
    last_err = None
    for attempt in range(5):
        try:
            res = run_bass_kernel_spmd(nc, in_maps, core_ids=list(range(8)))
            break
        except Exception as e:  # transient NRT/axon wedge: retry
            last_err = e
            import time
            time.sleep(20)
    else:
        raise last_err

    out = np.empty((B, S, D), dtype=np.float32)
    for b in range(B):
        p0 = np.asarray(res.results[2 * b]["out_part"]).astype(np.float32)
        p1_ = np.asarray(res.results[2 * b + 1]["out_part"]).astype(np.float32)
        tot = (p0 + p1_).reshape(D, S)  # [do, s]
        out[b] = tot.T + b_out[None, :]
    return out
